# revision 2
# baseline (speedup 1.0000x reference)
"""Causal self-attention (B=2, S=2048, D=2048, H=16) on 8 TRN2 NeuronCores.

Sharding: tensor-parallel over heads x data-parallel over batch.
Core c = b*4 + g handles batch b and heads 4g..4g+3 (head_dim=128).

Per-core device kernel (single NEFF, SPMD across 8 cores):
  stage 1: q/k projections into transposed layout qT/kT [d, S] (f32r matmuls),
           v projection into natural layout [S, d] stored bf16 with a fused
           ones-column per head (for softmax denominators).
  stage 2: per (head, 512-wide q-block): scores in transposed layout
           sT[j] = kT_j.T @ qT  (f32r, PSUM fp32); probs = exp(sT) -> bf16;
           diagonal-strip blocks multiplied by a static binary causal mask;
           av[q,128+1] = probs_j.T @ [v_j | 1] accumulated over j (bf16);
           row-normalize by reciprocal of the ones-column; PE-transpose to
           attoutT [d, q] (f32r).
  stage 3: y_partial = attoutT.T @ Wo_shard (f32r), DMA out fp32.

Softmax skips the max-subtraction (scores are O(5) for the expected input
distribution; a host-side sampling guard falls back to a numpy reference if
scores could overflow exp, or if the mask is not the canonical causal mask).

Host: y[b] = sum of the 4 per-core partials for that batch.
"""

import math
from contextlib import ExitStack

import numpy as np

B = 2
S = 2048
D = 2048
H = 16
HPC = 4  # heads per core
d = 128  # head dim
N_CORES = 8
P = 128
DK = D // P  # 16 contraction tiles
ST = S // P  # 16 token tiles
QB = S // 512  # 4 q-blocks of 512

_CACHE = {}


def _build_module():
    import concourse.mybir as mybir
    import concourse.tile as tile
    from concourse import bacc

    f32 = mybir.dt.float32
    f32r = mybir.dt.float32r
    bf16 = mybir.dt.bfloat16
    Exp = mybir.ActivationFunctionType.Exp

    nc = bacc.Bacc("TRN2", target_bir_lowering=False, debug=False)

    xT = nc.dram_tensor("xT", [D, S], f32r, kind="ExternalInput")
    wq = nc.dram_tensor("wq", [D, HPC * d], f32r, kind="ExternalInput")
    wk = nc.dram_tensor("wk", [D, HPC * d], f32r, kind="ExternalInput")
    wv = nc.dram_tensor("wv", [D, HPC * d], f32r, kind="ExternalInput")
    wo = nc.dram_tensor("wo", [HPC * d, D], f32r, kind="ExternalInput")
    bm = nc.dram_tensor("bm", [4, P, 512], bf16, kind="ExternalInput")
    ident = nc.dram_tensor("ident", [P, P], f32r, kind="ExternalInput")
    y = nc.dram_tensor("y", [S, D], f32, kind="ExternalOutput")

    xT_r = xT.ap().rearrange("(t p) s -> p t s", p=P)
    y_r = y.ap().rearrange("(t p) n -> p t n", p=P)

    with tile.TileContext(nc) as tc, ExitStack() as top:
        # pools that live across stages
        qkp = top.enter_context(tc.tile_pool(name="qkp", bufs=1))
        vp = top.enter_context(tc.tile_pool(name="vp", bufs=1))
        mp = top.enter_context(tc.tile_pool(name="mp", bufs=1))

        qT_sb = qkp.tile([P, HPC, S], f32r, tag="qT")
        kT_sb = qkp.tile([P, HPC, S], f32r, tag="kT")
        v_sb = vp.tile([P, ST, HPC, d + 1], bf16, tag="v")
        mask_sb = mp.tile([P, 4, 512], bf16, tag="bm")
        id_sb = mp.tile([P, P], f32r, tag="ident")

        nc.sync.dma_start(out=mask_sb, in_=bm.ap().rearrange("r p m -> p r m"))
        nc.sync.dma_start(out=id_sb, in_=ident.ap())
        nc.vector.memset(v_sb[:, :, :, d : d + 1], 1.0)

        # ---- stage 1a: q/k projections (tok chunks of 256) ----
        with ExitStack() as s1a:
            wqk = s1a.enter_context(tc.tile_pool(name="wqk", bufs=1))
            xap = s1a.enter_context(tc.tile_pool(name="xap", bufs=2))
            psa = s1a.enter_context(tc.tile_pool(name="psa", bufs=3, space="PSUM"))

            wq_sb = wqk.tile([P, DK, HPC * d], f32r, tag="wq")
            wk_sb = wqk.tile([P, DK, HPC * d], f32r, tag="wk")
            nc.sync.dma_start(out=wq_sb, in_=wq.ap().rearrange("(t p) m -> p t m", p=P))
            nc.sync.dma_start(out=wk_sb, in_=wk.ap().rearrange("(t p) m -> p t m", p=P))

            for c in range(S // 256):
                xa = xap.tile([P, DK, 256], f32r, tag="xa")
                nc.sync.dma_start(out=xa, in_=xT_r[:, :, c * 256 : (c + 1) * 256])
                for w_sb, dest in ((wq_sb, qT_sb), (wk_sb, kT_sb)):
                    for m in range(HPC):
                        ps = psa.tile([P, 256], f32, tag="pa")
                        for kk in range(DK):
                            nc.tensor.matmul(
                                ps,
                                w_sb[:, kk, m * d : (m + 1) * d],
                                xa[:, kk, :],
                                start=(kk == 0),
                                stop=(kk == DK - 1),
                            )
                        nc.vector.tensor_copy(
                            dest[:, m, c * 256 : (c + 1) * 256], ps
                        )

        # ---- stage 1b: v projection (tok chunks of 512) ----
        with ExitStack() as s1b:
            wvp = s1b.enter_context(tc.tile_pool(name="wvp", bufs=1))
            xbp = s1b.enter_context(tc.tile_pool(name="xbp", bufs=2))
            psb = s1b.enter_context(tc.tile_pool(name="psb", bufs=3, space="PSUM"))

            wv_sb = wvp.tile([P, DK, HPC * d], f32r, tag="wv")
            nc.sync.dma_start(out=wv_sb, in_=wv.ap().rearrange("(t p) m -> p t m", p=P))

            for c in range(S // 512):
                xb = xbp.tile([P, DK, 512], f32r, tag="xb")
                nc.sync.dma_start(out=xb, in_=xT_r[:, :, c * 512 : (c + 1) * 512])
                for mt in range(4):
                    ps = psb.tile([P, 512], f32, tag="pb")
                    for kk in range(DK):
                        nc.tensor.matmul(
                            ps,
                            xb[:, kk, mt * P : (mt + 1) * P],
                            wv_sb[:, kk, :],
                            start=(kk == 0),
                            stop=(kk == DK - 1),
                        )
                    ti = c * 4 + mt
                    nc.vector.tensor_copy(
                        v_sb[:, ti, :, 0:d],
                        ps.rearrange("p (h e) -> p h e", h=HPC),
                    )

        # pools for stages 2-3 open only after stage-1 pools are released
        aop = top.enter_context(tc.tile_pool(name="aop", bufs=1))
        wop = top.enter_context(tc.tile_pool(name="wop", bufs=1))
        attoutT_sb = aop.tile([P, HPC, S], f32r, tag="attoutT")
        wo_sb = wop.tile([P, HPC, D], f32r, tag="wo")
        # load wo while attention runs
        nc.sync.dma_start(out=wo_sb, in_=wo.ap().rearrange("(t p) n -> p t n", p=P))

        # ---- stage 2: attention ----
        with ExitStack() as s2:
            probp = s2.enter_context(tc.tile_pool(name="probp", bufs=20))
            smallp = s2.enter_context(tc.tile_pool(name="smallp", bufs=3))
            ps_sc = s2.enter_context(tc.tile_pool(name="ps_sc", bufs=3, space="PSUM"))
            ps_av = s2.enter_context(tc.tile_pool(name="ps_av", bufs=3, space="PSUM"))
            ps_tr = s2.enter_context(tc.tile_pool(name="ps_tr", bufs=2, space="PSUM"))

            for h in range(HPC):
                for Q in range(QB):
                    NK = 4 * Q + 4
                    probs = []
                    for j in range(NK):
                        sc = ps_sc.tile([P, 512], f32, tag="sc")
                        nc.tensor.matmul(
                            sc,
                            kT_sb[:, h, j * P : (j + 1) * P],
                            qT_sb[:, h, Q * 512 : (Q + 1) * 512],
                            start=True,
                            stop=True,
                        )
                        pj = probp.tile([P, 512], bf16, tag="probs")
                        nc.scalar.activation(out=pj, in_=sc, func=Exp)
                        r = j - 4 * Q
                        if r >= 0:
                            nc.vector.tensor_mul(pj, pj, mask_sb[:, r, :])
                        probs.append(pj)
                    for qt in range(4):
                        i = 4 * Q + qt
                        av = ps_av.tile([P, d + 1], f32, tag="av")
                        for j in range(i + 1):
                            nc.tensor.matmul(
                                av,
                                probs[j][:, qt * P : (qt + 1) * P],
                                v_sb[:, j, h, :],
                                start=(j == 0),
                                stop=(j == i),
                            )
                        rec = smallp.tile([P, 1], f32, tag="rec")
                        nc.vector.reciprocal(rec, av[:, d : d + 1])
                        ao = smallp.tile([P, d], f32r, tag="ao")
                        nc.vector.tensor_scalar_mul(ao, av[:, 0:d], rec)
                        pst = ps_tr.tile([P, P], f32r, tag="tr")
                        nc.tensor.transpose(pst, ao, id_sb)
                        nc.vector.tensor_copy(
                            attoutT_sb[:, h, i * P : (i + 1) * P], pst
                        )

        # ---- stage 3: output projection ----
        with ExitStack() as s3:
            ps3 = s3.enter_context(tc.tile_pool(name="ps3", bufs=3, space="PSUM"))
            yp = s3.enter_context(tc.tile_pool(name="yp", bufs=3))
            for qt in range(ST):
                for nchunk in range(4):
                    ps = ps3.tile([P, 512], f32, tag="py")
                    for h in range(HPC):
                        nc.tensor.matmul(
                            ps,
                            attoutT_sb[:, h, qt * P : (qt + 1) * P],
                            wo_sb[:, h, nchunk * 512 : (nchunk + 1) * 512],
                            start=(h == 0),
                            stop=(h == HPC - 1),
                        )
                    yt = yp.tile([P, 512], f32, tag="y")
                    nc.vector.tensor_copy(yt, ps)
                    nc.sync.dma_start(
                        out=y_r[:, qt, nchunk * 512 : (nchunk + 1) * 512], in_=yt
                    )

    nc.compile()
    return nc


def _static_inputs():
    import ml_dtypes

    masks = np.zeros((4, P, 512), dtype=np.float32)
    kk = np.arange(P)[:, None]
    qq = np.arange(512)[None, :]
    for r in range(4):
        masks[r] = (P * r + kk <= qq).astype(np.float32)
    return masks.astype(ml_dtypes.bfloat16), np.eye(P, dtype=np.float32)


def make_in_maps(x, Wq, Wk, Wv, Wo):
    """Shard full inputs into 8 per-core input dicts."""
    bm, ident = _static_inputs()
    scale = 1.0 / math.sqrt(d)
    in_maps = []
    for c in range(N_CORES):
        b, g = divmod(c, 4)
        hs = g * HPC * d  # 512*g: rows of Wq for this head group
        in_maps.append(
            {
                "xT": np.ascontiguousarray(x[b].T),
                "wq": np.ascontiguousarray(Wq[hs : hs + 512, :].T) * np.float32(scale),
                "wk": np.ascontiguousarray(Wk[hs : hs + 512, :].T),
                "wv": np.ascontiguousarray(Wv[hs : hs + 512, :].T),
                "wo": np.ascontiguousarray(Wo[:, hs : hs + 512].T),
                "bm": bm,
                "ident": ident,
            }
        )
    return in_maps


def combine_results(results):
    """results: list of 8 dicts with 'y' [S, D] partials -> full [B, S, D]."""
    y = np.zeros((B, S, D), dtype=np.float32)
    for c in range(N_CORES):
        b = c // 4
        y[b] += results[c]["y"]
    return y


def _is_canonical_causal(attn_mask):
    m = np.asarray(attn_mask).reshape(S, S)
    iu = np.triu_indices(S, k=1)
    if not np.all(m[iu] <= -1e8):
        return False
    il = np.tril_indices(S, k=0)
    return np.all(m[il] == 0.0)


def _scores_safe(x, Wq, Wk):
    """Sampled bound on |scores| to make exp-without-max safe."""
    rng = np.random.default_rng(0)
    qi = rng.choice(S, 96, replace=False)
    ki = rng.choice(S, 384, replace=False)
    mx = 0.0
    for b in range(B):
        q = (x[b][qi] @ Wq.T) / math.sqrt(d)  # [96, D]
        k = x[b][ki] @ Wk.T  # [384, D]
        qh = q.reshape(96, H, d)
        kh = k.reshape(384, H, d)
        s = np.einsum("qhd,khd->hqk", qh, kh)
        mx = max(mx, float(np.abs(s).max()))
    return mx < 30.0


def _numpy_reference(x, attn_mask, Wq, Wk, Wv, Wo):
    out = np.zeros((B, S, D), dtype=np.float32)
    m = np.asarray(attn_mask, dtype=np.float32).reshape(S, S)
    for b in range(B):
        q = (x[b] @ Wq.T).reshape(S, H, d).transpose(1, 0, 2)
        k = (x[b] @ Wk.T).reshape(S, H, d).transpose(1, 0, 2)
        v = (x[b] @ Wv.T).reshape(S, H, d).transpose(1, 0, 2)
        q = q / np.float32(math.sqrt(d))
        att_out = np.zeros((H, S, d), dtype=np.float32)
        for h in range(H):
            s = q[h] @ k[h].T + m
            s = s - s.max(axis=-1, keepdims=True)
            p = np.exp(s)
            p /= p.sum(axis=-1, keepdims=True)
            att_out[h] = p @ v[h]
        out[b] = att_out.transpose(1, 0, 2).reshape(S, D) @ Wo.T
    return out


def kernel(x, attn_mask, Wq, Wk, Wv, Wo):
    x = np.asarray(x, dtype=np.float32)
    Wq = np.asarray(Wq, dtype=np.float32)
    Wk = np.asarray(Wk, dtype=np.float32)
    Wv = np.asarray(Wv, dtype=np.float32)
    Wo = np.asarray(Wo, dtype=np.float32)

    if not _is_canonical_causal(attn_mask) or not _scores_safe(x, Wq, Wk):
        return _numpy_reference(x, attn_mask, Wq, Wk, Wv, Wo)

    from concourse.bass_utils import run_bass_kernel_spmd

    if "nc" not in _CACHE:
        _CACHE["nc"] = _build_module()
    nc = _CACHE["nc"]

    in_maps = make_in_maps(x, Wq, Wk, Wv, Wo)
    res = run_bass_kernel_spmd(nc, in_maps, core_ids=list(range(N_CORES)))
    return combine_results(res.results)


# revision 10
# speedup vs baseline: 100.3893x; 100.3893x over previous
"""Causal self-attention (B=2, S=2048, D=2048, H=16) on 8 TRN2 NeuronCores.

Sharding: tensor-parallel over heads x data-parallel over batch.
Core c = b*4 + g handles batch b and heads 4g..4g+3 (head_dim=128).

Per-core device kernel (single NEFF, SPMD across 8 cores):
  stage 1: q/k projections into transposed layout qT/kT [d, S] (f32r matmuls),
           v projection into natural layout [S, d] stored bf16 with a fused
           ones-column per head (for softmax denominators).
  stage 2: per (head, 512-wide q-block): scores in transposed layout
           sT[j] = kT_j.T @ qT  (f32r, PSUM fp32); probs = exp(sT) -> bf16;
           diagonal-strip blocks multiplied by a static binary causal mask;
           av[q,128+1] = probs_j.T @ [v_j | 1] accumulated over j (bf16);
           row-normalize by reciprocal of the ones-column; PE-transpose to
           attoutT [d, q] (f32r).
  stage 3: y_partial = attoutT.T @ Wo_shard (f32r), DMA out fp32.

Softmax skips the max-subtraction (scores are O(5) for the expected input
distribution; a host-side sampling guard falls back to a numpy reference if
scores could overflow exp, or if the mask is not the canonical causal mask).

Host: y[b] = sum of the 4 per-core partials for that batch.
"""

import math
from contextlib import ExitStack

import numpy as np

B = 2
S = 2048
D = 2048
H = 16
HPC = 4  # heads per core
d = 128  # head dim
N_CORES = 8
P = 128
DK = D // P  # 16 contraction tiles
ST = S // P  # 16 token tiles
QB = S // 512  # 4 q-blocks of 512

_CACHE = {}


def _build_module(repeat=1):
    import concourse.mybir as mybir
    import concourse.tile as tile
    from concourse import bacc

    f32 = mybir.dt.float32
    f32r = mybir.dt.float32r
    bf16 = mybir.dt.bfloat16
    Exp = mybir.ActivationFunctionType.Exp

    nc = bacc.Bacc("TRN2", target_bir_lowering=False, debug=False)

    xT = nc.dram_tensor("xT", [D, S], f32r, kind="ExternalInput")
    wq = nc.dram_tensor("wq", [D, HPC * d], f32r, kind="ExternalInput")
    wk = nc.dram_tensor("wk", [D, HPC * d], f32r, kind="ExternalInput")
    wv = nc.dram_tensor("wv", [D, HPC * d], f32r, kind="ExternalInput")
    wo = nc.dram_tensor("wo", [HPC * d, D], f32r, kind="ExternalInput")
    bm = nc.dram_tensor("bm", [4, P, 512], bf16, kind="ExternalInput")
    ident = nc.dram_tensor("ident", [P, P], f32r, kind="ExternalInput")
    y = nc.dram_tensor("y", [S, D], f32, kind="ExternalOutput")

    xT_r = xT.ap().rearrange("(t p) s -> p t s", p=P)
    y_r = y.ap().rearrange("(t p) n -> p t n", p=P)

    with tile.TileContext(nc) as tc, ExitStack() as top:
        # pools that live across stages
        qkp = top.enter_context(tc.tile_pool(name="qkp", bufs=1))
        vp = top.enter_context(tc.tile_pool(name="vp", bufs=1))
        mp = top.enter_context(tc.tile_pool(name="mp", bufs=1))

        qT_sb = qkp.tile([P, HPC, S], f32r, tag="qT")
        kT_sb = qkp.tile([P, HPC, S], f32r, tag="kT")
        v_sb = vp.tile([P, ST, HPC, d + 1], bf16, tag="v")
        mask_sb = mp.tile([P, 4, 512], bf16, tag="bm")
        id_sb = mp.tile([P, P], f32r, tag="ident")

        nc.vector.memset(v_sb[:, :, :, d : d + 1], 1.0)

        for _rep in range(repeat):
            wv_r = wv.ap().rearrange("(t p) m -> p t m", p=P)
            wv_pc = []
            s1x = ExitStack()
            wvpA = s1x.enter_context(tc.tile_pool(name="wvpA", bufs=1))

            # ---- stage 1a: q/k projections (tok chunks of 256) ----
            with ExitStack() as s1a:
                wqk = s1a.enter_context(tc.tile_pool(name="wqk", bufs=1))
                xap = s1a.enter_context(tc.tile_pool(name="xap", bufs=2))
                psa = s1a.enter_context(tc.tile_pool(name="psa", bufs=3, space="PSUM"))

                wq_r = wq.ap().rearrange("(t p) m -> p t m", p=P)
                wk_r = wk.ap().rearrange("(t p) m -> p t m", p=P)
                wq_pc, wk_pc = [], []
                for piece in range(4):
                    kk0, kk1 = piece * 4, (piece + 1) * 4
                    tq = wqk.tile([P, 4, HPC * d], f32r, tag=f"wq{piece}")
                    tk = wqk.tile([P, 4, HPC * d], f32r, tag=f"wk{piece}")
                    nc.sync.dma_start(out=tq, in_=wq_r[:, kk0:kk1, :])
                    nc.sync.dma_start(out=tk, in_=wk_r[:, kk0:kk1, :])
                    wq_pc.append(tq)
                    wk_pc.append(tk)
                # prefetch first half of wv behind wq/wk on the SP ring so
                # stage 1b can start without waiting for stage-1a space release
                for piece in range(2):
                    kk0, kk1 = piece * 4, (piece + 1) * 4
                    t = wvpA.tile([P, 4, HPC * d], f32r, tag=f"wv{piece}")
                    nc.sync.dma_start(out=t, in_=wv_r[:, kk0:kk1, :])
                    wv_pc.append(t)

                for c in range(S // 256):
                    xa_pc = []
                    for piece in range(4):
                        kk0, kk1 = piece * 4, (piece + 1) * 4
                        t = xap.tile([P, 4, 256], f32r, tag=f"xa{piece}")
                        nc.scalar.dma_start(
                            out=t, in_=xT_r[:, kk0:kk1, c * 256 : (c + 1) * 256]
                        )
                        xa_pc.append(t)
                    for w_pc, dest in ((wq_pc, qT_sb), (wk_pc, kT_sb)):
                        for m in range(HPC):
                            ps = psa.tile([P, 256], f32, tag="pa")
                            for kk in range(DK):
                                nc.tensor.matmul(
                                    ps,
                                    w_pc[kk // 4][:, kk % 4, m * d : (m + 1) * d],
                                    xa_pc[kk // 4][:, kk % 4, :],
                                    start=(kk == 0),
                                    stop=(kk == DK - 1),
                                )
                            nc.vector.tensor_copy(
                                dest[:, m, c * 256 : (c + 1) * 256], ps
                            )

            # ---- stage 1b: v projection (tok chunks of 512) ----
            with ExitStack() as s1b:
                wvp = s1b.enter_context(tc.tile_pool(name="wvp", bufs=1))
                xbp = s1b.enter_context(tc.tile_pool(name="xbp", bufs=2))
                psb = s1b.enter_context(tc.tile_pool(name="psb", bufs=3, space="PSUM"))

                for piece in range(2, 4):
                    kk0, kk1 = piece * 4, (piece + 1) * 4
                    t = wvp.tile([P, 4, HPC * d], f32r, tag=f"wv{piece}")
                    nc.sync.dma_start(out=t, in_=wv_r[:, kk0:kk1, :])
                    wv_pc.append(t)

                for c in range(S // 512):
                    xb_pc = []
                    for piece in range(4):
                        kk0, kk1 = piece * 4, (piece + 1) * 4
                        t = xbp.tile([P, 4, 512], f32r, tag=f"xb{piece}")
                        nc.scalar.dma_start(
                            out=t, in_=xT_r[:, kk0:kk1, c * 512 : (c + 1) * 512]
                        )
                        xb_pc.append(t)
                    for mt in range(4):
                        ps = psb.tile([P, 512], f32, tag="pb")
                        for kk in range(DK):
                            nc.tensor.matmul(
                                ps,
                                xb_pc[kk // 4][:, kk % 4, mt * P : (mt + 1) * P],
                                wv_pc[kk // 4][:, kk % 4, :],
                                start=(kk == 0),
                                stop=(kk == DK - 1),
                            )
                        ti = c * 4 + mt
                        nc.vector.tensor_copy(
                            v_sb[:, ti, :, 0:d],
                            ps.rearrange("p (h e) -> p h e", h=HPC),
                        )

            s1x.close()
            # pools for stages 2-3 open only after stage-1 pools are released
            s23 = ExitStack()
            aop = s23.enter_context(tc.tile_pool(name="aop", bufs=1))
            wop = s23.enter_context(tc.tile_pool(name="wop", bufs=1))
            attoutT_sb = aop.tile([P, HPC, S], f32r, tag="attoutT")
            wo_sb = wop.tile([P, HPC, D], f32r, tag="wo")
            # load wo + stage-2 constants while attention runs
            nc.sync.dma_start(out=wo_sb, in_=wo.ap().rearrange("(t p) n -> p t n", p=P))
            if _rep == 0:
                nc.scalar.dma_start(
                    out=mask_sb, in_=bm.ap().rearrange("r p m -> p r m")
                )
                nc.scalar.dma_start(out=id_sb, in_=ident.ap())

            # shared by stage-2 scores and stage-3 out-proj (overlap enabler)
            ps_sc = s23.enter_context(tc.tile_pool(name="ps_sc", bufs=3, space="PSUM"))

            # ---- stage 2: attention ----
            with ExitStack() as s2:
                probp = s2.enter_context(tc.tile_pool(name="probp", bufs=24))
                smallp = s2.enter_context(tc.tile_pool(name="smallp", bufs=3))
                ps_av = s2.enter_context(tc.tile_pool(name="ps_av", bufs=3, space="PSUM"))
                ps_tr = s2.enter_context(tc.tile_pool(name="ps_tr", bufs=2, space="PSUM"))

                for h in range(HPC):
                    for Q in range(QB):
                        NK = 4 * Q + 4
                        probs = []
                        for j in range(NK):
                            sc = ps_sc.tile([P, 512], f32, tag="sc")
                            nc.tensor.matmul(
                                sc,
                                kT_sb[:, h, j * P : (j + 1) * P],
                                qT_sb[:, h, Q * 512 : (Q + 1) * 512],
                                start=True,
                                stop=True,
                            )
                            pj = probp.tile([P, 512], bf16, tag="probs")
                            nc.scalar.activation(out=pj, in_=sc, func=Exp)
                            r = j - 4 * Q
                            if r >= 0:
                                nc.vector.tensor_mul(pj, pj, mask_sb[:, r, :])
                            probs.append(pj)
                        pst = ps_tr.tile([P, 4, P], f32r, tag="tr")
                        for qt in range(4):
                            i = 4 * Q + qt
                            av = ps_av.tile([P, d + 1], f32, tag="av")
                            for j in range(i + 1):
                                nc.tensor.matmul(
                                    av,
                                    probs[j][:, qt * P : (qt + 1) * P],
                                    v_sb[:, j, h, :],
                                    start=(j == 0),
                                    stop=(j == i),
                                )
                            rec = smallp.tile([P, 1], f32, tag="rec")
                            nc.vector.reciprocal(rec, av[:, d : d + 1])
                            ao = smallp.tile([P, d], f32r, tag="ao")
                            nc.vector.tensor_scalar_mul(ao, av[:, 0:d], rec)
                            nc.tensor.transpose(pst[:, qt, :], ao, id_sb)
                        nc.vector.tensor_copy(
                            attoutT_sb[:, h, Q * 512 : (Q + 1) * 512],
                            pst.rearrange("p q e -> p (q e)"),
                        )

            # ---- stage 3: output projection ----
            with ExitStack() as s3:
                yp = s3.enter_context(tc.tile_pool(name="yp", bufs=4))
                for qt in range(ST):
                    for nchunk in range(4):
                        ps = ps_sc.tile([P, 512], f32, tag="sc")
                        for h in range(HPC):
                            nc.tensor.matmul(
                                ps,
                                attoutT_sb[:, h, qt * P : (qt + 1) * P],
                                wo_sb[:, h, nchunk * 512 : (nchunk + 1) * 512],
                                start=(h == 0),
                                stop=(h == HPC - 1),
                            )
                        yt = yp.tile([P, 512], f32, tag="y")
                        nc.scalar.copy(yt, ps)
                        nc.sync.dma_start(
                            out=y_r[:, qt, nchunk * 512 : (nchunk + 1) * 512], in_=yt
                        )
            s23.close()

    nc.compile()
    return nc


def _static_inputs():
    import ml_dtypes

    masks = np.zeros((4, P, 512), dtype=np.float32)
    kk = np.arange(P)[:, None]
    qq = np.arange(512)[None, :]
    for r in range(4):
        masks[r] = (P * r + kk <= qq).astype(np.float32)
    return masks.astype(ml_dtypes.bfloat16), np.eye(P, dtype=np.float32)


def make_in_maps(x, Wq, Wk, Wv, Wo):
    """Shard full inputs into 8 per-core input dicts."""
    bm, ident = _static_inputs()
    scale = 1.0 / math.sqrt(d)
    in_maps = []
    for c in range(N_CORES):
        b, g = divmod(c, 4)
        hs = g * HPC * d  # 512*g: rows of Wq for this head group
        in_maps.append(
            {
                "xT": np.ascontiguousarray(x[b].T),
                "wq": np.ascontiguousarray(Wq[hs : hs + 512, :].T) * np.float32(scale),
                "wk": np.ascontiguousarray(Wk[hs : hs + 512, :].T),
                "wv": np.ascontiguousarray(Wv[hs : hs + 512, :].T),
                "wo": np.ascontiguousarray(Wo[:, hs : hs + 512].T),
                "bm": bm,
                "ident": ident,
            }
        )
    return in_maps


def combine_results(results):
    """results: list of 8 dicts with 'y' [S, D] partials -> full [B, S, D]."""
    y = np.zeros((B, S, D), dtype=np.float32)
    for c in range(N_CORES):
        b = c // 4
        y[b] += results[c]["y"]
    return y


def _is_canonical_causal(attn_mask):
    m = np.asarray(attn_mask).reshape(S, S)
    iu = np.triu_indices(S, k=1)
    if not np.all(m[iu] <= -1e8):
        return False
    il = np.tril_indices(S, k=0)
    return np.all(m[il] == 0.0)


def _scores_safe(x, Wq, Wk):
    """Sampled bound on |scores| to make exp-without-max safe."""
    rng = np.random.default_rng(0)
    qi = rng.choice(S, 96, replace=False)
    ki = rng.choice(S, 384, replace=False)
    mx = 0.0
    for b in range(B):
        q = (x[b][qi] @ Wq.T) / math.sqrt(d)  # [96, D]
        k = x[b][ki] @ Wk.T  # [384, D]
        qh = q.reshape(96, H, d)
        kh = k.reshape(384, H, d)
        s = np.einsum("qhd,khd->hqk", qh, kh)
        mx = max(mx, float(np.abs(s).max()))
    return mx < 30.0


def _numpy_reference(x, attn_mask, Wq, Wk, Wv, Wo):
    out = np.zeros((B, S, D), dtype=np.float32)
    m = np.asarray(attn_mask, dtype=np.float32).reshape(S, S)
    for b in range(B):
        q = (x[b] @ Wq.T).reshape(S, H, d).transpose(1, 0, 2)
        k = (x[b] @ Wk.T).reshape(S, H, d).transpose(1, 0, 2)
        v = (x[b] @ Wv.T).reshape(S, H, d).transpose(1, 0, 2)
        q = q / np.float32(math.sqrt(d))
        att_out = np.zeros((H, S, d), dtype=np.float32)
        for h in range(H):
            s = q[h] @ k[h].T + m
            s = s - s.max(axis=-1, keepdims=True)
            p = np.exp(s)
            p /= p.sum(axis=-1, keepdims=True)
            att_out[h] = p @ v[h]
        out[b] = att_out.transpose(1, 0, 2).reshape(S, D) @ Wo.T
    return out


def kernel(x, attn_mask, Wq, Wk, Wv, Wo):
    x = np.asarray(x, dtype=np.float32)
    Wq = np.asarray(Wq, dtype=np.float32)
    Wk = np.asarray(Wk, dtype=np.float32)
    Wv = np.asarray(Wv, dtype=np.float32)
    Wo = np.asarray(Wo, dtype=np.float32)

    if not _is_canonical_causal(attn_mask) or not _scores_safe(x, Wq, Wk):
        return _numpy_reference(x, attn_mask, Wq, Wk, Wv, Wo)

    from concourse.bass_utils import run_bass_kernel_spmd

    if "nc" not in _CACHE:
        _CACHE["nc"] = _build_module()
    nc = _CACHE["nc"]

    in_maps = make_in_maps(x, Wq, Wk, Wv, Wo)
    res = run_bass_kernel_spmd(nc, in_maps, core_ids=list(range(N_CORES)))
    return combine_results(res.results)



# revision 11
# speedup vs baseline: 102.7028x; 1.0230x over previous
"""Causal self-attention (B=2, S=2048, D=2048, H=16) on 8 TRN2 NeuronCores.

Sharding: tensor-parallel over heads x data-parallel over batch.
Core c = b*4 + g handles batch b and heads 4g..4g+3 (head_dim=128).

Per-core device kernel (single NEFF, SPMD across 8 cores):
  stage 1: q/k projections into transposed layout qT/kT [d, S] (f32r matmuls),
           v projection into natural layout [S, d] stored bf16 with a fused
           ones-column per head (for softmax denominators).
  stage 2: per (head, 512-wide q-block): scores in transposed layout
           sT[j] = kT_j.T @ qT  (f32r, PSUM fp32); probs = exp(sT) -> bf16;
           diagonal-strip blocks multiplied by a static binary causal mask;
           av[q,128+1] = probs_j.T @ [v_j | 1] accumulated over j (bf16);
           row-normalize by reciprocal of the ones-column; PE-transpose to
           attoutT [d, q] (f32r).
  stage 3: y_partial = attoutT.T @ Wo_shard (f32r), DMA out fp32.

Softmax skips the max-subtraction (scores are O(5) for the expected input
distribution; a host-side sampling guard falls back to a numpy reference if
scores could overflow exp, or if the mask is not the canonical causal mask).

Host: y[b] = sum of the 4 per-core partials for that batch.
"""

import math
from contextlib import ExitStack

import numpy as np

B = 2
S = 2048
D = 2048
H = 16
HPC = 4  # heads per core
d = 128  # head dim
N_CORES = 8
P = 128
DK = D // P  # 16 contraction tiles
ST = S // P  # 16 token tiles
QB = S // 512  # 4 q-blocks of 512

_CACHE = {}


def _build_module(repeat=1):
    import concourse.mybir as mybir
    import concourse.tile as tile
    from concourse import bacc

    f32 = mybir.dt.float32
    f32r = mybir.dt.float32r
    bf16 = mybir.dt.bfloat16
    Exp = mybir.ActivationFunctionType.Exp

    nc = bacc.Bacc("TRN2", target_bir_lowering=False, debug=False)

    xT = nc.dram_tensor("xT", [D, S], f32r, kind="ExternalInput")
    wq = nc.dram_tensor("wq", [D, HPC * d], f32r, kind="ExternalInput")
    wk = nc.dram_tensor("wk", [D, HPC * d], f32r, kind="ExternalInput")
    wv = nc.dram_tensor("wv", [D, HPC * d], f32r, kind="ExternalInput")
    wo = nc.dram_tensor("wo", [HPC * d, D], f32r, kind="ExternalInput")
    bm = nc.dram_tensor("bm", [4, P, 512], bf16, kind="ExternalInput")
    ident = nc.dram_tensor("ident", [P, P], f32r, kind="ExternalInput")
    y = nc.dram_tensor("y", [S, D], f32, kind="ExternalOutput")

    xT_r = xT.ap().rearrange("(t p) s -> p t s", p=P)
    y_r = y.ap().rearrange("(t p) n -> p t n", p=P)

    with tile.TileContext(nc) as tc, ExitStack() as top:
        # pools that live across stages
        qkp = top.enter_context(tc.tile_pool(name="qkp", bufs=1))
        vp = top.enter_context(tc.tile_pool(name="vp", bufs=1))
        mp = top.enter_context(tc.tile_pool(name="mp", bufs=1))

        qT_sb = qkp.tile([P, HPC, S], f32r, tag="qT")
        kT_sb = qkp.tile([P, HPC, S], f32r, tag="kT")
        v_sb = vp.tile([P, ST, HPC, d + 1], bf16, tag="v")
        mask_sb = mp.tile([P, 4, 512], bf16, tag="bm")
        id_sb = mp.tile([P, P], f32r, tag="ident")

        nc.vector.memset(v_sb[:, :, :, d : d + 1], 1.0)

        for _rep in range(repeat):
            wv_r = wv.ap().rearrange("(t p) m -> p t m", p=P)
            wv_pc = []
            s1x = ExitStack()
            wvpA = s1x.enter_context(tc.tile_pool(name="wvpA", bufs=1))

            # ---- stage 1a: q/k projections (tok chunks of 256) ----
            with ExitStack() as s1a:
                wqk = s1a.enter_context(tc.tile_pool(name="wqk", bufs=1))
                xap = s1a.enter_context(tc.tile_pool(name="xap", bufs=2))
                psa = s1a.enter_context(tc.tile_pool(name="psa", bufs=3, space="PSUM"))

                wq_r = wq.ap().rearrange("(t p) m -> p t m", p=P)
                wk_r = wk.ap().rearrange("(t p) m -> p t m", p=P)
                wq_pc, wk_pc = [], []
                for piece in range(4):
                    kk0, kk1 = piece * 4, (piece + 1) * 4
                    tq = wqk.tile([P, 4, HPC * d], f32r, tag=f"wq{piece}")
                    tk = wqk.tile([P, 4, HPC * d], f32r, tag=f"wk{piece}")
                    nc.sync.dma_start(out=tq, in_=wq_r[:, kk0:kk1, :])
                    wq_pc.append(tq)
                    wk_pc.append(tk)
                # prefetch first half of wv behind wq on the SP ring so
                # stage 1b can start without waiting for stage-1a space release
                for piece in range(2):
                    kk0, kk1 = piece * 4, (piece + 1) * 4
                    t = wvpA.tile([P, 4, HPC * d], f32r, tag=f"wv{piece}")
                    nc.sync.dma_start(out=t, in_=wv_r[:, kk0:kk1, :])
                    wv_pc.append(t)
                xb0 = wvpA.tile([P, 4, 512], f32r, tag="xb0pre")
                nc.sync.dma_start(out=xb0, in_=xT_r[:, 0:4, 0:512])

                for c in range(S // 256):
                    xa_pc = []
                    for piece in range(4):
                        kk0, kk1 = piece * 4, (piece + 1) * 4
                        t = xap.tile([P, 4, 256], f32r, tag=f"xa{piece}")
                        nc.scalar.dma_start(
                            out=t, in_=xT_r[:, kk0:kk1, c * 256 : (c + 1) * 256]
                        )
                        xa_pc.append(t)
                    if c == 0:
                        # wk rides the ACT ring right behind chunk-0 x pieces,
                        # in parallel with wq on the SP ring
                        for piece in range(4):
                            kk0, kk1 = piece * 4, (piece + 1) * 4
                            nc.scalar.dma_start(
                                out=wk_pc[piece], in_=wk_r[:, kk0:kk1, :]
                            )
                    for w_pc, dest in ((wq_pc, qT_sb), (wk_pc, kT_sb)):
                        for m in range(HPC):
                            ps = psa.tile([P, 256], f32, tag="pa")
                            for kk in range(DK):
                                nc.tensor.matmul(
                                    ps,
                                    w_pc[kk // 4][:, kk % 4, m * d : (m + 1) * d],
                                    xa_pc[kk // 4][:, kk % 4, :],
                                    start=(kk == 0),
                                    stop=(kk == DK - 1),
                                )
                            nc.vector.tensor_copy(
                                dest[:, m, c * 256 : (c + 1) * 256], ps
                            )

            # ---- stage 1b: v projection (tok chunks of 512) ----
            with ExitStack() as s1b:
                wvp = s1b.enter_context(tc.tile_pool(name="wvp", bufs=1))
                xbp = s1b.enter_context(tc.tile_pool(name="xbp", bufs=2))
                psb = s1b.enter_context(tc.tile_pool(name="psb", bufs=3, space="PSUM"))

                for piece in range(2, 4):
                    kk0, kk1 = piece * 4, (piece + 1) * 4
                    t = wvp.tile([P, 4, HPC * d], f32r, tag=f"wv{piece}")
                    nc.sync.dma_start(out=t, in_=wv_r[:, kk0:kk1, :])
                    wv_pc.append(t)

                for c in range(S // 512):
                    xb_pc = []
                    for piece in range(4):
                        if c == 0 and piece == 0:
                            xb_pc.append(xb0)
                            continue
                        kk0, kk1 = piece * 4, (piece + 1) * 4
                        t = xbp.tile([P, 4, 512], f32r, tag=f"xb{piece}")
                        nc.scalar.dma_start(
                            out=t, in_=xT_r[:, kk0:kk1, c * 512 : (c + 1) * 512]
                        )
                        xb_pc.append(t)
                    for mt in range(4):
                        ps = psb.tile([P, 512], f32, tag="pb")
                        for kk in range(DK):
                            nc.tensor.matmul(
                                ps,
                                xb_pc[kk // 4][:, kk % 4, mt * P : (mt + 1) * P],
                                wv_pc[kk // 4][:, kk % 4, :],
                                start=(kk == 0),
                                stop=(kk == DK - 1),
                            )
                        ti = c * 4 + mt
                        nc.vector.tensor_copy(
                            v_sb[:, ti, :, 0:d],
                            ps.rearrange("p (h e) -> p h e", h=HPC),
                        )

            s1x.close()
            # pools for stages 2-3 open only after stage-1 pools are released
            s23 = ExitStack()
            aop = s23.enter_context(tc.tile_pool(name="aop", bufs=1))
            wop = s23.enter_context(tc.tile_pool(name="wop", bufs=1))
            attoutT_sb = aop.tile([P, HPC, S], f32r, tag="attoutT")
            wo_sb = wop.tile([P, HPC, D], f32r, tag="wo")
            # load wo + stage-2 constants while attention runs
            nc.sync.dma_start(out=wo_sb, in_=wo.ap().rearrange("(t p) n -> p t n", p=P))
            if _rep == 0:
                nc.scalar.dma_start(
                    out=mask_sb, in_=bm.ap().rearrange("r p m -> p r m")
                )
                nc.scalar.dma_start(out=id_sb, in_=ident.ap())

            # shared by stage-2 scores and stage-3 out-proj (overlap enabler)
            ps_sc = s23.enter_context(tc.tile_pool(name="ps_sc", bufs=3, space="PSUM"))

            # ---- stage 2: attention ----
            with ExitStack() as s2:
                probp = s2.enter_context(tc.tile_pool(name="probp", bufs=24))
                smallp = s2.enter_context(tc.tile_pool(name="smallp", bufs=3))
                ps_av = s2.enter_context(tc.tile_pool(name="ps_av", bufs=3, space="PSUM"))
                ps_tr = s2.enter_context(tc.tile_pool(name="ps_tr", bufs=2, space="PSUM"))

                for h in range(HPC):
                    for Q in range(QB):
                        NK = 4 * Q + 4
                        probs = []
                        for j in range(NK):
                            sc = ps_sc.tile([P, 512], f32, tag="sc")
                            nc.tensor.matmul(
                                sc,
                                kT_sb[:, h, j * P : (j + 1) * P],
                                qT_sb[:, h, Q * 512 : (Q + 1) * 512],
                                start=True,
                                stop=True,
                            )
                            pj = probp.tile([P, 512], bf16, tag="probs")
                            nc.scalar.activation(out=pj, in_=sc, func=Exp)
                            r = j - 4 * Q
                            if r >= 0:
                                nc.vector.tensor_mul(pj, pj, mask_sb[:, r, :])
                            probs.append(pj)
                        pst = ps_tr.tile([P, 4, P], f32r, tag="tr")
                        for qt in range(4):
                            i = 4 * Q + qt
                            av = ps_av.tile([P, d + 1], f32, tag="av")
                            for j in range(i + 1):
                                nc.tensor.matmul(
                                    av,
                                    probs[j][:, qt * P : (qt + 1) * P],
                                    v_sb[:, j, h, :],
                                    start=(j == 0),
                                    stop=(j == i),
                                )
                            rec = smallp.tile([P, 1], f32, tag="rec")
                            nc.vector.reciprocal(rec, av[:, d : d + 1])
                            ao = smallp.tile([P, d], f32r, tag="ao")
                            nc.vector.tensor_scalar_mul(ao, av[:, 0:d], rec)
                            nc.tensor.transpose(pst[:, qt, :], ao, id_sb)
                        nc.vector.tensor_copy(
                            attoutT_sb[:, h, Q * 512 : (Q + 1) * 512],
                            pst.rearrange("p q e -> p (q e)"),
                        )

            # ---- stage 3: output projection ----
            with ExitStack() as s3:
                yp = s3.enter_context(tc.tile_pool(name="yp", bufs=4))
                for qt in range(ST):
                    for nchunk in range(4):
                        ps = ps_sc.tile([P, 512], f32, tag="sc")
                        for h in range(HPC):
                            nc.tensor.matmul(
                                ps,
                                attoutT_sb[:, h, qt * P : (qt + 1) * P],
                                wo_sb[:, h, nchunk * 512 : (nchunk + 1) * 512],
                                start=(h == 0),
                                stop=(h == HPC - 1),
                            )
                        yt = yp.tile([P, 512], f32, tag="y")
                        nc.scalar.copy(yt, ps)
                        nc.sync.dma_start(
                            out=y_r[:, qt, nchunk * 512 : (nchunk + 1) * 512], in_=yt
                        )
            s23.close()

    nc.compile()
    return nc


def _static_inputs():
    import ml_dtypes

    masks = np.zeros((4, P, 512), dtype=np.float32)
    kk = np.arange(P)[:, None]
    qq = np.arange(512)[None, :]
    for r in range(4):
        masks[r] = (P * r + kk <= qq).astype(np.float32)
    return masks.astype(ml_dtypes.bfloat16), np.eye(P, dtype=np.float32)


def make_in_maps(x, Wq, Wk, Wv, Wo):
    """Shard full inputs into 8 per-core input dicts."""
    bm, ident = _static_inputs()
    scale = 1.0 / math.sqrt(d)
    in_maps = []
    for c in range(N_CORES):
        b, g = divmod(c, 4)
        hs = g * HPC * d  # 512*g: rows of Wq for this head group
        in_maps.append(
            {
                "xT": np.ascontiguousarray(x[b].T),
                "wq": np.ascontiguousarray(Wq[hs : hs + 512, :].T) * np.float32(scale),
                "wk": np.ascontiguousarray(Wk[hs : hs + 512, :].T),
                "wv": np.ascontiguousarray(Wv[hs : hs + 512, :].T),
                "wo": np.ascontiguousarray(Wo[:, hs : hs + 512].T),
                "bm": bm,
                "ident": ident,
            }
        )
    return in_maps


def combine_results(results):
    """results: list of 8 dicts with 'y' [S, D] partials -> full [B, S, D]."""
    y = np.zeros((B, S, D), dtype=np.float32)
    for c in range(N_CORES):
        b = c // 4
        y[b] += results[c]["y"]
    return y


def _is_canonical_causal(attn_mask):
    m = np.asarray(attn_mask).reshape(S, S)
    iu = np.triu_indices(S, k=1)
    if not np.all(m[iu] <= -1e8):
        return False
    il = np.tril_indices(S, k=0)
    return np.all(m[il] == 0.0)


def _scores_safe(x, Wq, Wk):
    """Sampled bound on |scores| to make exp-without-max safe."""
    rng = np.random.default_rng(0)
    qi = rng.choice(S, 96, replace=False)
    ki = rng.choice(S, 384, replace=False)
    mx = 0.0
    for b in range(B):
        q = (x[b][qi] @ Wq.T) / math.sqrt(d)  # [96, D]
        k = x[b][ki] @ Wk.T  # [384, D]
        qh = q.reshape(96, H, d)
        kh = k.reshape(384, H, d)
        s = np.einsum("qhd,khd->hqk", qh, kh)
        mx = max(mx, float(np.abs(s).max()))
    return mx < 30.0


def _numpy_reference(x, attn_mask, Wq, Wk, Wv, Wo):
    out = np.zeros((B, S, D), dtype=np.float32)
    m = np.asarray(attn_mask, dtype=np.float32).reshape(S, S)
    for b in range(B):
        q = (x[b] @ Wq.T).reshape(S, H, d).transpose(1, 0, 2)
        k = (x[b] @ Wk.T).reshape(S, H, d).transpose(1, 0, 2)
        v = (x[b] @ Wv.T).reshape(S, H, d).transpose(1, 0, 2)
        q = q / np.float32(math.sqrt(d))
        att_out = np.zeros((H, S, d), dtype=np.float32)
        for h in range(H):
            s = q[h] @ k[h].T + m
            s = s - s.max(axis=-1, keepdims=True)
            p = np.exp(s)
            p /= p.sum(axis=-1, keepdims=True)
            att_out[h] = p @ v[h]
        out[b] = att_out.transpose(1, 0, 2).reshape(S, D) @ Wo.T
    return out


def kernel(x, attn_mask, Wq, Wk, Wv, Wo):
    x = np.asarray(x, dtype=np.float32)
    Wq = np.asarray(Wq, dtype=np.float32)
    Wk = np.asarray(Wk, dtype=np.float32)
    Wv = np.asarray(Wv, dtype=np.float32)
    Wo = np.asarray(Wo, dtype=np.float32)

    if not _is_canonical_causal(attn_mask) or not _scores_safe(x, Wq, Wk):
        return _numpy_reference(x, attn_mask, Wq, Wk, Wv, Wo)

    from concourse.bass_utils import run_bass_kernel_spmd

    if "nc" not in _CACHE:
        _CACHE["nc"] = _build_module()
    nc = _CACHE["nc"]

    in_maps = make_in_maps(x, Wq, Wk, Wv, Wo)
    res = run_bass_kernel_spmd(nc, in_maps, core_ids=list(range(N_CORES)))
    return combine_results(res.results)

